# revision 11
# baseline (speedup 1.0000x reference)
"""Two-layer GAT (PyG-style, eval mode) on 8 Trainium2 NeuronCores.

Strategy (dst-sharded, per-node edge columns):
  - Host: shard destination nodes into 8 contiguous ranges (one per core).
    Within a core, nodes are permuted by in-degree so that each window of 128
    nodes has near-uniform degree; window w stores its edges as K[w] columns
    of 128 rows, where row p holds the in-edges of the window's node p
    (padded to K[w] = max in-degree in the window, ~12% overhead).
    Self-loops are excluded from the edge lists (their features are local).
  - Device phase A: per-core rows of  rec1 = [x @ W1 + b1 | alpha_src | alpha_dst]
    via PE matmuls (alpha vectors folded into the weight matrix:
    (x@W1)@A = x@(W1@A)), then AllGather so every core holds the full table.
  - Device phase B (layer-1 edge phase, per window): one dma_gather of node
    records by edge source; alpha_dst is a plain per-partition value from the
    local record rows; ex = exp(leaky_relu(a_s+a_d)) computed without the
    segment-max shift (edge logits are bounded ~[-2,2] here, so this is
    numerically exact).  Edge-column pads gather a dedicated pad row whose
    alpha_src is -200, so they contribute exp(-40) ~ 4e-18 -- no mask ops.
    The segment scatter-add reduces over the K edge columns with a single
    strided tensor_reduce per window (no matmuls); self-loop terms are added
    from the local rows; normalize, ELU, and produce layer-2 records
    rec2 = [h @ W2 | alpha2_src | alpha2_dst].
  - AllGather rec2; phase D repeats the edge phase for layer 2 (1 head,
    40 channels), buffering per-window logits on chip; log_softmax runs as
    one batched tail pass (single ACT table load, one Ln, one broadcast
    subtract, one strided output DMA).  The host inverse-permutes the rows.
    Windows are processed largest-degree-first so each phase drains on its
    cheapest window.

    Layer-1 records are 768 B: 256 bf16 features + 16 fp32 alphas + pad
    (gather elements must be 256B-multiples); the feature rounding costs
    ~6e-5 relative error end to end.  Measured bottleneck: SWDGE descriptor
    emission at ~8.7 ns per gathered row (invariant to packeting and element
    size), ~718 us of the ~1.07 ms total; the rest is the phase-A/AllGather
    head and pipeline ramp.

The only cross-core traffic is the two AllGathers of the node tables.  The
dominant device cost is SWDGE descriptor emission for the by-source gathers
(~9 ns/edge), which this layout minimizes: one gathered row per non-self
edge per layer, nothing else.
"""

import sys

for _p in ("/opt/trn_rl_repo", "/root/.axon_site/_ro/trn_rl_repo"):
    if _p not in sys.path:
        sys.path.append(_p)

import numpy as np

import concourse.bass as bass
import concourse.mybir as mybir
import concourse.tile as tile
from concourse import bacc
from concourse.bass_utils import run_bass_kernel_spmd
from concourse.masks import make_identity

F32 = mybir.dt.float32
I16 = mybir.dt.int16
ALU = mybir.AluOpType
ACTF = mybir.ActivationFunctionType
AXX = mybir.AxisListType.X

CORES = 8
PW = 128           # nodes per window
NEG_SLOPE = 0.2
SPLIT_CC = False   # split AllGathers in half (Local tables) vs single (Shared)

_CACHE = {}


# --------------------------------------------------------------------------
# host-side preprocessing
# --------------------------------------------------------------------------

def _wrap_idx(vals):
    """Wrap a flat index vector into the [128, n/16] layout dma_gather wants
    (index i at partition i%16, col i//16; replicated to all 8 Q7 groups)."""
    w = vals.reshape(-1, 16).T.astype(np.int16)
    return np.ascontiguousarray(np.tile(w, (8, 1)))


def _prep(x, edge_index, W1, a1_src, a1_dst, b1, W2, a2_src, a2_dst, b2):
    N, F = x.shape
    H, C = a1_src.shape
    OUT = W2.shape[1]
    NPC = N // CORES
    W = (NPC + PW - 1) // PW
    NPCP = W * PW
    if NPCP == NPC:
        NPCP += PW          # spare pad block (holds the -200 pad row)

    src0 = np.asarray(edge_index[0], np.int64)
    dst0 = np.asarray(edge_index[1], np.int64)

    deg = np.zeros(N, np.int64)
    np.add.at(deg, dst0, 1)

    # per-core degree-sorted node permutation; perm[new_global_row] = node id
    perm = np.empty(N, np.int64)
    for c in range(CORES):
        ids = np.arange(c * NPC, (c + 1) * NPC)
        perm[c * NPC:(c + 1) * NPC] = ids[np.argsort(deg[ids], kind="stable")]
    newrow = np.empty(N, np.int64)
    newrow[perm] = np.arange(N)          # node id -> permuted global row
    # tab1 rows: plain [core, NPCP] (single Shared AllGather);
    # tab2 rows: [half, core, NPCP//2] so its AllGather can split.
    HHALF = NPCP
    HH2 = NPCP // 2 if NPCP % (2 * PW) == 0 else NPCP
    _c = newrow // NPC
    _r = newrow % NPC
    tabrow = _c * NPCP + _r
    tabrow2 = (_r // HH2) * (CORES * HH2) + _c * HH2 + (_r % HH2)

    # per-window max degree (shared across cores for SPMD uniformity)
    degw = deg[perm].reshape(CORES, NPC)
    K = []
    for w in range(W):
        lo, hi = w * PW, min((w + 1) * PW, NPC)
        K.append(max(1, int(degw[:, lo:hi].max())))
    K = np.asarray(K, np.int64)
    CK = np.concatenate([[0], np.cumsum(K)])   # column offsets
    SK = int(K.sum())
    EPC = SK * PW                              # gather indices per core/layer

    # in-edges grouped by (permuted) destination row: CSR over new rows
    order = np.argsort(newrow[dst0], kind="stable")
    src_s = src0[order]
    starts = np.zeros(N + 1, np.int64)
    np.add.at(starts, newrow[dst0] + 1, 1)
    starts = np.cumsum(starts)

    isrc = np.zeros((CORES, EPC), np.int64)
    isrc2 = np.zeros((CORES, EPC), np.int64)
    for c in range(CORES):
        # pads gather the core's first pad row (alpha_src overwritten to -200)
        isrc[c, :] = c * NPCP + NPC
        isrc2[c, :] = ((NPC // HH2) * (CORES * HH2) + c * HH2 + (NPC % HH2))
        for w in range(W):
            kw = int(K[w])
            for p in range(PW):
                lp = w * PW + p
                if lp >= NPC:
                    continue
                r = c * NPC + lp
                s, e = starts[r], starts[r + 1]
                d = e - s
                cols = CK[w] + np.arange(d)
                isrc[c, cols * PW + p] = tabrow[src_s[s:e]]
                isrc2[c, cols * PW + p] = tabrow2[src_s[s:e]]

    isrc_w = np.stack([_wrap_idx(isrc[c]) for c in range(CORES)])
    isrc2_w = np.stack([_wrap_idx(isrc2[c]) for c in range(CORES)])

    # folded weight matrices (f64 for exactness of the tiny folds)
    As = np.zeros((F, H), np.float64)
    Ad = np.zeros((F, H), np.float64)
    for h in range(H):
        As[h * C:(h + 1) * C, h] = a1_src[h]
        Ad[h * C:(h + 1) * C, h] = a1_dst[h]
    W1_64 = np.asarray(W1, np.float64)
    wc1 = np.concatenate(
        [np.asarray(W1, np.float32),
         (W1_64 @ As).astype(np.float32),
         (W1_64 @ Ad).astype(np.float32)], 1)           # [F, F+2H]
    W2_64 = np.asarray(W2, np.float64)
    wc2 = np.concatenate(
        [np.asarray(W2, np.float32),
         (W2_64 @ np.asarray(a2_src[0], np.float64))[:, None].astype(np.float32),
         (W2_64 @ np.asarray(a2_dst[0], np.float64))[:, None].astype(np.float32)],
        1)                                               # [F, OUT+2]

    KT = F // 128
    xt = np.zeros((CORES, KT, 128, NPCP), np.float32)
    xp = np.asarray(x, np.float32)[perm]
    for c in range(CORES):
        xt[c, :, :, :NPC] = (xp[c * NPC:(c + 1) * NPC].T
                             .reshape(KT, 128, NPC))

    host = {
        "N": N, "F": F, "H": H, "C": C, "OUT": OUT,
        "NPC": NPC, "W": W, "NPCP": NPCP,
        "K": tuple(int(k) for k in K), "SK": SK, "EPC": EPC,
        "HHALF": HHALF, "HH2": HH2, "perm": perm,
        "use_b1": bool(np.any(b1)), "use_b2": bool(np.any(b2)),
    }
    in_maps = []
    for c in range(CORES):
        m = {
            "xt": xt[c],
            "wc1": np.ascontiguousarray(wc1.reshape(2, F // 2, F + 2 * H)),
            "wc2": np.ascontiguousarray(wc2.reshape(2, F // 2, OUT + 2)),
            "isrc": isrc_w[c],
            "isrc2": isrc2_w[c],
        }
        if host["use_b1"]:
            m["b1rep"] = np.broadcast_to(np.asarray(b1, np.float32), (PW, F)).copy()
        if host["use_b2"]:
            m["b2rep"] = np.broadcast_to(np.asarray(b2, np.float32), (PW, OUT)).copy()
        in_maps.append(m)
    return host, in_maps


# --------------------------------------------------------------------------
# device kernel
# --------------------------------------------------------------------------

def _build(hp):
    F, H, C, OUT = hp["F"], hp["H"], hp["C"], hp["OUT"]
    W, NPCP = hp["W"], hp["NPCP"]
    K, SK, EPC = hp["K"], hp["SK"], hp["EPC"]
    KMAX = max(K)
    KT = F // 128               # K tiles (2)
    BF16 = mybir.dt.bfloat16
    # layer-1 record, 768 bytes: [256 x bf16 features | 16 x f32 alphas | pad]
    REC1 = 2 * F + 64 + 64      # in bf16 units: 256 feat + 32 (16 f32) + 96 pad
    REC1 = F + 32 + 96          # 384 bf16 units = 768 B
    assert (REC1 * 2) % 256 == 0
    A1 = F + 32                 # valid bf16 cols (feat + alpha area)
    AF = F + 2 * H              # 272 f32 matmul output cols
    REC2 = 64                   # OUT+2 valid cols of rec2
    A2 = OUT + 2
    NTAB = CORES * NPCP
    EPC16 = EPC // 16
    HHALF = hp["HHALF"]
    WB = HHALF // PW            # windows per table half
    HH2 = hp["HH2"]
    WB2 = HH2 // PW
    NPC = hp["NPC"]
    CK = [0]
    for k in K:
        CK.append(CK[-1] + k)

    nc = bacc.Bacc(None, target_bir_lowering=False, num_swdge_queues=4)

    xt_p = nc.declare_dram_parameter("xt", [KT, 128, NPCP], F32, isOutput=False)
    wc1_p = nc.declare_dram_parameter("wc1", [KT, 128, AF], F32, isOutput=False)
    wc2_p = nc.declare_dram_parameter("wc2", [KT, 128, A2], F32, isOutput=False)
    isrc_p = nc.declare_dram_parameter("isrc", [128, EPC16], I16, isOutput=False)
    isrc2_p = nc.declare_dram_parameter("isrc2", [128, EPC16], I16, isOutput=False)
    b1_p = (nc.declare_dram_parameter("b1rep", [PW, F], F32, isOutput=False)
            if hp["use_b1"] else None)
    b2_p = (nc.declare_dram_parameter("b2rep", [PW, OUT], F32, isOutput=False)
            if hp["use_b2"] else None)
    out_p = nc.declare_dram_parameter("out", [NPCP, OUT], F32, isOutput=True)

    with tile.TileContext(nc) as tc:
        with (
            tc.tile_pool(name="dram", bufs=1, space="DRAM") as dram,
            tc.tile_pool(name="const", bufs=1) as cpool,
            tc.tile_pool(name="io", bufs=3) as iop,
            tc.tile_pool(name="gath", bufs=4) as gp,
            tc.tile_pool(name="mid", bufs=2) as mp,
            tc.tile_pool(name="psA", bufs=4, space="PSUM") as psA,
            tc.tile_pool(name="psB", bufs=2, space="PSUM") as psB,
        ):
            r1loc = dram.tile([NPCP, REC1], BF16)
            tab1 = dram.tile([NTAB, REC1], BF16,
                             addr_space="Local" if HHALF < NPCP else "Shared")
            r2loc = dram.tile([NPCP, REC2], F32)
            tab2 = dram.tile([NTAB, REC2], F32,
                             addr_space="Local" if HH2 < NPCP else "Shared")

            # resident constants
            wc1_sb = cpool.tile([128, KT, AF], F32)
            for g in range(KT):
                nc.sync.dma_start(out=wc1_sb[:, g, :], in_=wc1_p[g])
            wc2_sb = cpool.tile([128, KT, A2], F32)
            for g in range(KT):
                nc.sync.dma_start(out=wc2_sb[:, g, :], in_=wc2_p[g])
            ident = cpool.tile([PW, PW], F32)
            make_identity(nc, ident[:])
            isrc_sb = cpool.tile([128, EPC16], I16)
            nc.sync.dma_start(out=isrc_sb[:], in_=isrc_p[:])
            isrc2_sb = cpool.tile([128, EPC16], I16)
            nc.sync.dma_start(out=isrc2_sb[:], in_=isrc2_p[:])
            if b1_p is not None:
                b1_sb = cpool.tile([PW, F], F32)
                nc.sync.dma_start(out=b1_sb[:], in_=b1_p[:])
            if b2_p is not None:
                b2_sb = cpool.tile([PW, OUT], F32)
                nc.sync.dma_start(out=b2_sb[:], in_=b2_p[:])
            # zero the table pad columns once (keeps gathered bytes finite)
            zt = cpool.tile([PW, REC1 - A1], BF16)
            nc.vector.memset(zt[:], 0.0)
            zt2 = cpool.tile([PW, REC2 - A2], F32)
            nc.vector.memset(zt2[:], 0.0)
            pcA = cpool.tile([PW, 16], F32)      # pad-row alphas for rec1
            nc.vector.memset(pcA[:], 0.0)
            nc.vector.memset(pcA[:, :H], -200.0)
            pzA = cpool.tile([PW, F], BF16)      # pad-row features (zero)
            nc.vector.memset(pzA[:], 0.0)
            pcB = cpool.tile([PW, 2], F32)       # pad-row alphas for rec2
            nc.vector.memset(pcB[:], 0.0)
            nc.vector.memset(pcB[:, :1], -200.0)
            pzB = cpool.tile([PW, OUT], F32)
            nc.vector.memset(pzB[:], 0.0)
            lgall = cpool.tile([128, W, OUT], F32)
            ezsc = cpool.tile([128, OUT], F32)
            ssall = cpool.tile([128, W], F32)
            lsall = cpool.tile([128, W], F32)
            for nt in range(W):
                nc.sync.dma_start(out=r1loc[nt * PW:(nt + 1) * PW, A1:], in_=zt[:])
                nc.sync.dma_start(out=r2loc[nt * PW:(nt + 1) * PW, A2:], in_=zt2[:])

            # ---------------- phase A: rec1 rows for this core ------------
            for nt in range(W):
                xw = mp.tile([128, KT, PW], F32, tag="xw")
                for g in range(KT):
                    nc.sync.dma_start(out=xw[:, g, :],
                                      in_=xt_p[g][:, nt * PW:(nt + 1) * PW])
                rp = psB.tile([128, AF], F32, tag="acc")
                for g in range(KT):
                    nc.tensor.matmul(rp[:], lhsT=xw[:, g, :],
                                     rhs=wc1_sb[:, g, :],
                                     start=(g == 0), stop=(g == KT - 1))
                rsf = mp.tile([128, 2 * H], F32, tag="rsf")   # alpha cols, f32
                nc.vector.tensor_copy(out=rsf[:], in_=rp[:, F:])
                rsb = mp.tile([128, F], BF16, tag="rsb")      # features, bf16
                if b1_p is not None:
                    # bias folded into the aggregated features (sum alpha = 1)
                    nc.vector.tensor_add(out=rsb[:], in0=rp[:, :F], in1=b1_sb[:])
                else:
                    nc.vector.tensor_copy(out=rsb[:], in_=rp[:, :F])
                nc.sync.dma_start(out=r1loc[nt * PW:(nt + 1) * PW, :F], in_=rsb[:])
                nc.sync.dma_start(
                    out=r1loc[nt * PW:(nt + 1) * PW, F:F + 32].bitcast(F32),
                    in_=rsf[:])
                if HHALF < NPCP and nt == WB - 1:
                    nc.gpsimd.collective_compute(
                        "AllGather", ALU.bypass,
                        replica_groups=[list(range(CORES))],
                        ins=[r1loc[:HHALF, :].opt()],
                        outs=[tab1[:CORES * HHALF, :].opt()])
            # pad rows: zero features, alpha_src = -200 (=> exp ~ 4e-18)
            for lo in range(NPC, NPCP, PW):
                nr = min(PW, NPCP - lo)
                nc.sync.dma_start(out=r1loc[lo:lo + nr, :F], in_=pzA[:nr])
                nc.sync.dma_start(
                    out=r1loc[lo:lo + nr, F:F + 32].bitcast(F32), in_=pcA[:nr])
            nc.gpsimd.collective_compute(
                "AllGather", ALU.bypass, replica_groups=[list(range(CORES))],
                ins=[r1loc[HHALF - HHALF:, :].opt() if HHALF == NPCP
                     else r1loc[HHALF:, :].opt()],
                outs=[tab1[:, :].opt() if HHALF == NPCP
                      else tab1[CORES * HHALF:, :].opt()])

            # ---------------- phase B: layer-1 edge phase -----------------
            for w in reversed(range(W)):
                KW = K[w]
                NI = KW * PW
                G1 = gp.tile([128, KW, REC1], BF16, tag="G1")
                nc.gpsimd.dma_gather(
                    G1[:], tab1[:, :], isrc_sb[:, CK[w] * 8:(CK[w] + KW) * 8],
                    NI, NI, REC1, single_packet=False, queue_num=w % 4)
                loc = mp.tile([128, A1], BF16, tag="loc")
                nc.sync.dma_start(out=loc[:], in_=r1loc[w * PW:(w + 1) * PW, :A1])
                loc_as = loc[:, F:F + 16].bitcast(F32)        # [128, 8] f32
                loc_ad = loc[:, F + 16:F + 32].bitcast(F32)   # [128, 8] f32
                # ex = exp(leaky_relu(a_s[src] + a_d[dst]))  (pads -> ~0)
                es_t = mp.tile([128, KMAX, H], F32, tag="es")
                es = es_t[:, :KW, :]
                nc.vector.tensor_tensor(
                    out=es, in0=G1[:, :, F:F + 16].bitcast(F32),
                    in1=loc_ad.unsqueeze(1).to_broadcast([128, KW, H]),
                    op=ALU.add)
                nc.vector.scalar_tensor_tensor(
                    out=es, in0=es, scalar=NEG_SLOPE, in1=es,
                    op0=ALU.mult, op1=ALU.max)
                nc.scalar.activation(out=es, in_=es, func=ACTF.Exp)
                # self-loop term
                ess = mp.tile([128, H], F32, tag="ess")
                nc.vector.tensor_tensor(out=ess[:], in0=loc_as,
                                        in1=loc_ad, op=ALU.add)
                nc.vector.scalar_tensor_tensor(
                    out=ess[:], in0=ess[:], scalar=NEG_SLOPE, in1=ess[:],
                    op0=ALU.mult, op1=ALU.max)
                nc.scalar.activation(out=ess[:], in_=ess[:], func=ACTF.Exp)
                # denominator = sum_t ex + ex_self   (always >= ex_self > 0)
                den = mp.tile([128, H], F32, tag="den")
                nc.vector.tensor_reduce(
                    out=den[:], in_=es.rearrange("p t h -> p h t"),
                    axis=AXX, op=ALU.add)
                nc.vector.tensor_add(out=den[:], in0=den[:], in1=ess[:])
                rcp = mp.tile([128, H], F32, tag="rcp")
                nc.vector.reciprocal(rcp[:], den[:])
                # weighted feature sum over edge columns + self
                nc.vector.tensor_tensor(
                    out=G1[:, :, :F].rearrange("p t (h c) -> p t h c", h=H),
                    in0=G1[:, :, :F].rearrange("p t (h c) -> p t h c", h=H),
                    in1=es.unsqueeze(3).to_broadcast([128, KW, H, C]),
                    op=ALU.mult)
                num = mp.tile([128, F], F32, tag="num")
                nc.vector.tensor_reduce(
                    out=num[:], in_=G1[:, :, :F].rearrange("p t f -> p f t"),
                    axis=AXX, op=ALU.add)
                tmp = mp.tile([128, F], F32, tag="tmp")
                nc.vector.tensor_tensor(
                    out=tmp[:].rearrange("p (h c) -> p h c", h=H),
                    in0=loc[:, :F].rearrange("p (h c) -> p h c", h=H),
                    in1=ess[:].unsqueeze(2).to_broadcast([128, H, C]),
                    op=ALU.mult)
                nc.vector.tensor_add(out=num[:], in0=num[:], in1=tmp[:])
                ho = mp.tile([128, F], F32, tag="ho")
                nc.vector.tensor_tensor(
                    out=ho[:].rearrange("p (h c) -> p h c", h=H),
                    in0=num[:].rearrange("p (h c) -> p h c", h=H),
                    in1=rcp[:].unsqueeze(2).to_broadcast([128, H, C]),
                    op=ALU.mult)
                # ELU(x) = relu(x) + exp(min(x,0)) - 1
                xm = mp.tile([128, F], F32, tag="xm")
                nc.vector.tensor_scalar_min(out=xm[:], in0=ho[:], scalar1=0.0)
                nc.scalar.activation(out=xm[:], in_=xm[:], func=ACTF.Exp)
                nc.vector.tensor_scalar_max(out=ho[:], in0=ho[:], scalar1=0.0)
                nc.vector.scalar_tensor_tensor(
                    out=ho[:], in0=ho[:], scalar=-1.0, in1=xm[:],
                    op0=ALU.add, op1=ALU.add)
                hT = mp.tile([128, KT, 128], F32, tag="hT")
                for g in range(KT):
                    tp = psA.tile([128, 128], F32, tag="tp")
                    nc.tensor.transpose(out=tp[:], in_=ho[:, g * 128:(g + 1) * 128],
                                        identity=ident[:])
                    nc.vector.tensor_copy(out=hT[:, g, :], in_=tp[:])
                r2p = psB.tile([128, A2], F32, tag="acc2")
                for g in range(KT):
                    nc.tensor.matmul(r2p[:], lhsT=hT[:, g, :], rhs=wc2_sb[:, g, :],
                                     start=(g == 0), stop=(g == KT - 1))
                r2sb = mp.tile([128, A2], F32, tag="r2sb")
                nc.vector.tensor_copy(out=r2sb[:], in_=r2p[:])
                nc.sync.dma_start(out=r2loc[w * PW:(w + 1) * PW, :A2], in_=r2sb[:])
                if HH2 < NPCP and w == min(WB2, W - 1):
                    # upper half rows done (reversed order): pad rows + CC2b
                    for lo in range(NPC, NPCP, PW):
                        nr = min(PW, NPCP - lo)
                        nc.sync.dma_start(out=r2loc[lo:lo + nr, :OUT],
                                          in_=pzB[:nr])
                        nc.sync.dma_start(out=r2loc[lo:lo + nr, OUT:OUT + 2],
                                          in_=pcB[:nr])
                    nc.gpsimd.collective_compute(
                        "AllGather", ALU.bypass,
                        replica_groups=[list(range(CORES))],
                        ins=[r2loc[HH2:, :].opt()],
                        outs=[tab2[CORES * HH2:, :].opt()])

            if HH2 == NPCP:
                for lo in range(NPC, NPCP, PW):
                    nr = min(PW, NPCP - lo)
                    nc.sync.dma_start(out=r2loc[lo:lo + nr, :OUT], in_=pzB[:nr])
                    nc.sync.dma_start(out=r2loc[lo:lo + nr, OUT:OUT + 2],
                                      in_=pcB[:nr])
                nc.gpsimd.collective_compute(
                    "AllGather", ALU.bypass, replica_groups=[list(range(CORES))],
                    ins=[r2loc[:, :].opt()], outs=[tab2[:, :].opt()])
            else:
                nc.gpsimd.collective_compute(
                    "AllGather", ALU.bypass, replica_groups=[list(range(CORES))],
                    ins=[r2loc[:HH2, :].opt()],
                    outs=[tab2[:CORES * HH2, :].opt()])

            # ---------------- phase D: layer-2 edge phase -----------------
            for w in reversed(range(W)):
                KW = K[w]
                NI = KW * PW
                G2 = gp.tile([128, KW, REC2], F32, tag="G2")
                nc.gpsimd.dma_gather(
                    G2[:], tab2[:, :], isrc2_sb[:, CK[w] * 8:(CK[w] + KW) * 8],
                    NI, NI, REC2, single_packet=False, queue_num=w % 4)
                loc2 = mp.tile([128, A2], F32, tag="loc2")
                nc.sync.dma_start(out=loc2[:], in_=r2loc[w * PW:(w + 1) * PW, :A2])
                es2_t = mp.tile([128, KMAX, 1], F32, tag="es2")
                es2 = es2_t[:, :KW, :]
                nc.vector.tensor_tensor(
                    out=es2, in0=G2[:, :, OUT:OUT + 1],
                    in1=loc2[:, OUT + 1:OUT + 2].unsqueeze(1)
                        .to_broadcast([128, KW, 1]),
                    op=ALU.add)
                nc.vector.scalar_tensor_tensor(
                    out=es2, in0=es2, scalar=NEG_SLOPE, in1=es2,
                    op0=ALU.mult, op1=ALU.max)
                nc.scalar.activation(out=es2, in_=es2, func=ACTF.Exp)
                ess2 = mp.tile([128, 1], F32, tag="ess2")
                nc.vector.tensor_tensor(out=ess2[:], in0=loc2[:, OUT:OUT + 1],
                                        in1=loc2[:, OUT + 1:OUT + 2], op=ALU.add)
                nc.vector.scalar_tensor_tensor(
                    out=ess2[:], in0=ess2[:], scalar=NEG_SLOPE, in1=ess2[:],
                    op0=ALU.mult, op1=ALU.max)
                nc.scalar.activation(out=ess2[:], in_=ess2[:], func=ACTF.Exp)
                den2 = mp.tile([128, 1], F32, tag="den2")
                nc.vector.tensor_reduce(
                    out=den2[:], in_=es2.rearrange("p t h -> p h t"),
                    axis=AXX, op=ALU.add)
                nc.vector.tensor_add(out=den2[:], in0=den2[:], in1=ess2[:])
                rcp2 = mp.tile([128, 1], F32, tag="rcp2")
                nc.vector.reciprocal(rcp2[:], den2[:])
                nc.vector.tensor_tensor(
                    out=G2[:, :, :OUT], in0=G2[:, :, :OUT],
                    in1=es2.to_broadcast([128, KW, OUT]), op=ALU.mult)
                num2 = mp.tile([128, OUT], F32, tag="num2")
                nc.vector.tensor_reduce(
                    out=num2[:], in_=G2[:, :, :OUT].rearrange("p t f -> p f t"),
                    axis=AXX, op=ALU.add)
                tmp2 = mp.tile([128, OUT], F32, tag="tmp2")
                nc.vector.tensor_scalar_mul(out=tmp2[:], in0=loc2[:, :OUT],
                                            scalar1=ess2[:, :1])
                nc.vector.tensor_add(out=num2[:], in0=num2[:], in1=tmp2[:])
                nc.vector.tensor_scalar_mul(out=lgall[:, w, :], in0=num2[:],
                                            scalar1=rcp2[:, :1])
                if b2_p is not None:
                    nc.vector.tensor_add(out=lgall[:, w, :], in0=lgall[:, w, :],
                                         in1=b2_sb[:])

            # batched log_softmax over all windows (no max-shift: logits are
            # bounded ~[-2, 2]); one ACT table load for the 20 Exps, one Ln.
            for w in range(W):
                nc.scalar.activation(out=ezsc[:], in_=lgall[:, w, :],
                                     func=ACTF.Exp, accum_out=ssall[:, w:w + 1])
            nc.scalar.activation(out=lsall[:], in_=ssall[:], func=ACTF.Ln)
            nc.vector.tensor_tensor(
                out=lgall[:], in0=lgall[:],
                in1=lsall[:].unsqueeze(2).to_broadcast([128, W, OUT]),
                op=ALU.subtract)
            nc.sync.dma_start(
                out=out_p[:W * PW].rearrange("(w p) o -> p w o", p=PW),
                in_=lgall[:])

    nc.compile()
    return nc


# --------------------------------------------------------------------------
# public entry point
# --------------------------------------------------------------------------

def kernel(x, edge_index, W1, a1_src, a1_dst, b1, W2, a2_src, a2_dst, b2,
           _want_trace=False):
    x = np.asarray(x)
    host, in_maps = _prep(x, np.asarray(edge_index), np.asarray(W1),
                          np.asarray(a1_src), np.asarray(a1_dst),
                          np.asarray(b1), np.asarray(W2), np.asarray(a2_src),
                          np.asarray(a2_dst), np.asarray(b2))
    key = (host["N"], host["F"], host["H"], host["C"], host["OUT"],
           host["K"], host["use_b1"], host["use_b2"])
    if key not in _CACHE:
        _CACHE[key] = _build(host)
    nc = _CACHE[key]
    res = run_bass_kernel_spmd(nc, in_maps, core_ids=list(range(CORES)),
                               trace=_want_trace)
    NPC = host["NPC"]
    out = np.empty((host["N"], host["OUT"]), np.float32)
    for c in range(CORES):
        out[host["perm"][c * NPC:(c + 1) * NPC]] = res.results[c]["out"][:NPC]
    if _want_trace:
        kernel._last_result = res
    return np.ascontiguousarray(out)

